# revision 2
# baseline (speedup 1.0000x reference)
"""ConvLSTM cell (B=32, C_IN=32, HC=64, H=W=64, K=3) on 8 trn2 NeuronCores.

Strategy: data-parallel over batch (4 images per core), weights replicated.
Per image: stack x (32ch) + h (64ch) into one zero-padded bf16 SBUF tile
[96, 66*68]; the fused conv (4 x-convs + 4 h-convs -> 256 gate channels)
becomes 9 shifted matmuls per output-channel chunk accumulating in PSUM.
Gate chunks: chunk0 = [f, i], chunk1 = [o, g] so the LSTM elementwise math
runs at full 128-partition width where possible.
"""

import os
import sys

import numpy as np

if "/opt/trn_rl_repo" not in sys.path:
    sys.path.insert(0, "/opt/trn_rl_repo")

import ml_dtypes

BF16 = ml_dtypes.bfloat16

B, C_IN, HC, H, W, K = 32, 32, 64, 64, 64, 3
N_CORES = 8
B_LOC = B // N_CORES  # 4 images per core
CTOT = C_IN + HC  # 96 combined input channels
PW = 68  # padded row width (2 left, 2 right; interior cols 2..65)
PH = 66  # padded rows (1 top, 1 bottom; interior rows 1..64)
ROWS_PER_BLK = 16  # pixels per block = 16*64 = 1024
N_BLK = H // ROWS_PER_BLK  # 4 blocks per image
BLK_PX = ROWS_PER_BLK * W  # 1024
SUB_PX = 512  # one matmul / PSUM bank
SUB_ROWS = SUB_PX // W  # 8

_CACHE: dict = {}


def _build_program():
    import concourse.bacc as bacc
    import concourse.mybir as mybir
    import concourse.tile as tile

    nc = bacc.Bacc("TRN2", target_bir_lowering=False, debug=False)
    f32 = mybir.dt.float32
    bf16 = mybir.dt.bfloat16
    AF = mybir.ActivationFunctionType

    x_d = nc.dram_tensor("x", [B_LOC, C_IN, H, W], bf16, kind="ExternalInput").ap()
    h_d = nc.dram_tensor("h", [B_LOC, HC, H, W], bf16, kind="ExternalInput").ap()
    c_d = nc.dram_tensor("c", [B_LOC, HC, H, W], f32, kind="ExternalInput").ap()
    w_d = nc.dram_tensor("w", [CTOT, 9 * 4 * HC], bf16, kind="ExternalInput").ap()
    b_d = nc.dram_tensor("bias", [128, 2], f32, kind="ExternalInput").ap()
    cn_d = nc.dram_tensor("cn", [B_LOC, HC, H, W], f32, kind="ExternalOutput").ap()
    hn_d = nc.dram_tensor("hn", [B_LOC, HC, H, W], f32, kind="ExternalOutput").ap()

    with tile.TileContext(nc) as tc:
        with (
            tc.tile_pool(name="const", bufs=1) as constp,
            tc.tile_pool(name="pt", bufs=1) as ptp,
            tc.tile_pool(name="psum0", bufs=2, space="PSUM") as pp0,
            tc.tile_pool(name="psum1", bufs=2, space="PSUM") as pp1,
            tc.tile_pool(name="work", bufs=3) as sp,
        ):
            w_sb = constp.tile([CTOT, 9 * 4 * HC], bf16)
            nc.sync.dma_start(w_sb[:], w_d)
            b_sb = constp.tile([128, 2], f32)
            nc.sync.dma_start(b_sb[:], b_d)

            # two persistent padded input buffers, zeroed once; interior is
            # overwritten per image, borders stay zero
            pts = [
                ptp.tile([CTOT, PH * PW], bf16, tag=f"pt{i}", name=f"pt{i}")
                for i in range(2)
            ]
            for t in pts:
                nc.gpsimd.memset(t[:], 0.0)

            for b in range(B_LOC):
                pt = pts[b % 2]
                pt3 = pt[:].rearrange("c (y x) -> c y x", x=PW)
                nc.sync.dma_start(pt3[0:C_IN, 1 : H + 1, 2 : W + 2], x_d[b])
                nc.sync.dma_start(pt3[C_IN:CTOT, 1 : H + 1, 2 : W + 2], h_d[b])

                for blk in range(N_BLK):
                    y0 = blk * ROWS_PER_BLK
                    P0 = pp0.tile([128, BLK_PX], f32)
                    P1 = pp1.tile([128, BLK_PX], f32)
                    for chunk, P in ((0, P0), (1, P1)):
                        for off in range(9):
                            dy, dx = off // 3, off % 3
                            lo = off * 256 + chunk * 128
                            lhsT = w_sb[:, lo : lo + 128]
                            for sub in range(2):
                                r0 = y0 + sub * SUB_ROWS + dy
                                rhs = pt3[:, r0 : r0 + SUB_ROWS, dx + 1 : dx + 1 + W]
                                nc.tensor.matmul(
                                    P[:, sub * SUB_PX : (sub + 1) * SUB_PX],
                                    lhsT,
                                    rhs,
                                    start=(off == 0),
                                    stop=(off == 8),
                                )

                    # elementwise LSTM math for this 1024-px block
                    # P0 = [f | i], P1 = [o | g] (by 64-partition halves)
                    s_fi = sp.tile([128, BLK_PX], f32, tag="sfi")
                    nc.scalar.activation(
                        s_fi[:], P0[:], AF.Sigmoid, bias=b_sb[:, 0:1]
                    )
                    so = sp.tile([64, BLK_PX], f32, tag="so")
                    nc.scalar.activation(
                        so[:], P1[0:64, :], AF.Sigmoid, bias=b_sb[0:64, 1:2]
                    )
                    cg = sp.tile([128, BLK_PX], f32, tag="cg")
                    nc.scalar.activation(
                        cg[64:128, :], P1[64:128, :], AF.Tanh, bias=b_sb[64:128, 1:2]
                    )
                    nc.sync.dma_start(
                        cg[0:64, :].rearrange("c (y x) -> c y x", x=W),
                        c_d[b, :, y0 : y0 + ROWS_PER_BLK, :],
                    )
                    prod = sp.tile([128, BLK_PX], f32, tag="prod")
                    nc.vector.tensor_mul(prod[:], s_fi[:], cg[:])
                    ig = sp.tile([64, BLK_PX], f32, tag="ig")
                    nc.sync.dma_start(ig[:], prod[64:128, :])
                    cn = sp.tile([64, BLK_PX], f32, tag="cn")
                    nc.vector.tensor_add(cn[:], prod[0:64, :], ig[:])
                    tch = sp.tile([64, BLK_PX], f32, tag="tch")
                    nc.scalar.activation(tch[:], cn[:], AF.Tanh)
                    hh = sp.tile([64, BLK_PX], f32, tag="hh")
                    nc.vector.tensor_mul(hh[:], so[:], tch[:])
                    nc.scalar.dma_start(
                        cn_d[b, :, y0 : y0 + ROWS_PER_BLK, :],
                        cn[:].rearrange("c (y x) -> c y x", x=W),
                    )
                    nc.scalar.dma_start(
                        hn_d[b, :, y0 : y0 + ROWS_PER_BLK, :],
                        hh[:].rearrange("c (y x) -> c y x", x=W),
                    )

    nc.compile()
    return nc


def get_program():
    if "nc" not in _CACHE:
        _CACHE["nc"] = _build_program()
    return _CACHE["nc"]


def _prep_host(inputs):
    """Pack weights/biases; convert x/h to bf16; build per-core input maps."""
    x = np.asarray(inputs["x"], np.float32)
    h = np.asarray(inputs["hidden_state"], np.float32)
    c = np.asarray(inputs["cell_state"], np.float32)

    # gate column order [f, i, o, g] -> chunk0=[f,i], chunk1=[o,g]
    gx = [inputs["w_xf"], inputs["w_xi"], inputs["w_xo"], inputs["w_xg"]]
    gh = [inputs["w_hf"], inputs["w_hi"], inputs["w_ho"], inputs["w_hg"]]
    wx = np.stack([np.asarray(a, np.float32) for a in gx])  # [4, HC, C_IN, 3, 3]
    wh = np.stack([np.asarray(a, np.float32) for a in gh])  # [4, HC, HC, 3, 3]
    # -> [c, dy, dx, gate, o] -> [c, 9, 256]
    wxc = np.transpose(wx, (2, 3, 4, 0, 1)).reshape(C_IN, 9, 4 * HC)
    whc = np.transpose(wh, (2, 3, 4, 0, 1)).reshape(HC, 9, 4 * HC)
    wcat = np.concatenate([wxc, whc], 0).reshape(CTOT, 9 * 4 * HC).astype(BF16)

    bf = np.asarray(inputs["b_xf"], np.float32) + np.asarray(inputs["b_hf"], np.float32)
    bi = np.asarray(inputs["b_xi"], np.float32) + np.asarray(inputs["b_hi"], np.float32)
    bo = np.asarray(inputs["b_xo"], np.float32) + np.asarray(inputs["b_ho"], np.float32)
    bg = np.asarray(inputs["b_xg"], np.float32) + np.asarray(inputs["b_hg"], np.float32)
    bias = np.stack(
        [np.concatenate([bf, bi]), np.concatenate([bo, bg])], axis=1
    ).astype(np.float32)  # [128, 2]

    xb = x.astype(BF16)
    hb = h.astype(BF16)

    in_maps = []
    for i in range(N_CORES):
        s = slice(i * B_LOC, (i + 1) * B_LOC)
        in_maps.append(
            {
                "x": xb[s],
                "h": hb[s],
                "c": c[s],
                "w": wcat,
                "bias": bias,
            }
        )
    return in_maps


def run(inputs, trace=False, trace_kwargs=None):
    from concourse.bass_utils import run_bass_kernel_spmd

    nc = get_program()
    in_maps = _prep_host(inputs)
    res = run_bass_kernel_spmd(
        nc,
        in_maps,
        list(range(N_CORES)),
        trace=trace,
        **(trace_kwargs or {}),
    )
    h_new = np.concatenate([r["hn"] for r in res.results], 0).astype(np.float32)
    c_new = np.concatenate([r["cn"] for r in res.results], 0).astype(np.float32)
    return (h_new, c_new), res


def kernel(**inputs):
    (h_new, c_new), _ = run(inputs, trace=False)
    return (h_new, c_new)


# revision 3
# speedup vs baseline: 1.0060x; 1.0060x over previous
"""ConvLSTM cell (B=32, C_IN=32, HC=64, H=W=64, K=3) on 8 trn2 NeuronCores.

Strategy: data-parallel over batch (4 images per core), weights replicated.
Per image: stack x (32ch) + h (64ch) into one zero-padded bf16 SBUF tile
[96, 66*68]; the fused conv (4 x-convs + 4 h-convs -> 256 gate channels)
becomes 9 shifted matmuls per output-channel chunk accumulating in PSUM.
Gate chunks: chunk0 = [f, i], chunk1 = [o, g] so the LSTM elementwise math
runs at full 128-partition width where possible.
"""

import os
import sys

import numpy as np

if "/opt/trn_rl_repo" not in sys.path:
    sys.path.insert(0, "/opt/trn_rl_repo")

import ml_dtypes

BF16 = ml_dtypes.bfloat16

B, C_IN, HC, H, W, K = 32, 32, 64, 64, 64, 3
N_CORES = 8
B_LOC = B // N_CORES  # 4 images per core
CTOT = C_IN + HC  # 96 combined input channels
PW = 68  # padded row width (2 left, 2 right; interior cols 2..65)
PH = 66  # padded rows (1 top, 1 bottom; interior rows 1..64)
ROWS_PER_BLK = 16  # pixels per block = 16*64 = 1024
N_BLK = H // ROWS_PER_BLK  # 4 blocks per image
BLK_PX = ROWS_PER_BLK * W  # 1024
SUB_PX = 512  # one matmul / PSUM bank
SUB_ROWS = SUB_PX // W  # 8

_CACHE: dict = {}


def _build_program():
    import concourse.bacc as bacc
    import concourse.mybir as mybir
    import concourse.tile as tile

    nc = bacc.Bacc("TRN2", target_bir_lowering=False, debug=False)
    f32 = mybir.dt.float32
    bf16 = mybir.dt.bfloat16
    AF = mybir.ActivationFunctionType

    x_d = nc.dram_tensor("x", [B_LOC, C_IN, H, W], bf16, kind="ExternalInput").ap()
    h_d = nc.dram_tensor("h", [B_LOC, HC, H, W], bf16, kind="ExternalInput").ap()
    c_d = nc.dram_tensor("c", [B_LOC, HC, H, W], f32, kind="ExternalInput").ap()
    w_d = nc.dram_tensor("w", [CTOT, 9 * 4 * HC], bf16, kind="ExternalInput").ap()
    b_d = nc.dram_tensor("bias", [128, 2], f32, kind="ExternalInput").ap()
    cn_d = nc.dram_tensor("cn", [B_LOC, HC, H, W], f32, kind="ExternalOutput").ap()
    hn_d = nc.dram_tensor("hn", [B_LOC, HC, H, W], f32, kind="ExternalOutput").ap()

    with tile.TileContext(nc) as tc:
        with (
            tc.tile_pool(name="const", bufs=1) as constp,
            tc.tile_pool(name="pt", bufs=6) as ptp,
            tc.tile_pool(name="psum0", bufs=2, space="PSUM") as pp0,
            tc.tile_pool(name="psum1", bufs=2, space="PSUM") as pp1,
            tc.tile_pool(name="work", bufs=3) as sp,
        ):
            w_sb = constp.tile([CTOT, 9 * 4 * HC], bf16)
            nc.sync.dma_start(w_sb[:], w_d)
            b_sb = constp.tile([128, 2], f32)
            nc.sync.dma_start(b_sb[:], b_d)

            bi = 0
            for b in range(B_LOC):
                # last image uses half-size blocks so the end-of-kernel
                # elementwise drain is shorter
                rpb = 8 if b == B_LOC - 1 else ROWS_PER_BLK
                nblk = H // rpb
                nsub = rpb // SUB_ROWS
                for blk in range(nblk):
                    y0 = blk * rpb
                    nrows = rpb + 2  # one halo row each side
                    # per-block padded input tile: local row L = image row
                    # y0-1+L; cols 2..65 = image cols 0..63
                    pt = ptp.tile(
                        [CTOT, nrows * PW], bf16, tag="ptb", name=f"ptb{bi}"
                    )
                    pt3 = pt[:].rearrange("c (y x) -> c y x", x=PW)
                    nc.gpsimd.memset(pt3[:, :, 0:2], 0.0)
                    nc.gpsimd.memset(pt3[:, :, W + 2 : PW], 0.0)
                    gs = max(0, y0 - 1)
                    ge = min(H, y0 + rpb + 1)
                    ls = gs - (y0 - 1)
                    le = ge - (y0 - 1)
                    if ls > 0:
                        nc.gpsimd.memset(pt3[:, 0:ls, 2 : W + 2], 0.0)
                    if le < nrows:
                        nc.gpsimd.memset(pt3[:, le:nrows, 2 : W + 2], 0.0)
                    nc.sync.dma_start(
                        pt3[0:C_IN, ls:le, 2 : W + 2], x_d[b, :, gs:ge, :]
                    )
                    nc.sync.dma_start(
                        pt3[C_IN:CTOT, ls:le, 2 : W + 2], h_d[b, :, gs:ge, :]
                    )

                    blk_px = rpb * W
                    P0 = pp0.tile([128, blk_px], f32, tag="P0", name=f"P0_{bi}")
                    P1 = pp1.tile([128, blk_px], f32, tag="P1", name=f"P1_{bi}")
                    for chunk, P in ((0, P0), (1, P1)):
                        for off in range(9):
                            dy, dx = off // 3, off % 3
                            lo = off * 256 + chunk * 128
                            lhsT = w_sb[:, lo : lo + 128]
                            for sub in range(nsub):
                                r0 = sub * SUB_ROWS + dy
                                rhs = pt3[:, r0 : r0 + SUB_ROWS, dx + 1 : dx + 1 + W]
                                nc.tensor.matmul(
                                    P[:, sub * SUB_PX : (sub + 1) * SUB_PX],
                                    lhsT,
                                    rhs,
                                    start=(off == 0),
                                    stop=(off == 8),
                                )

                    # elementwise LSTM math for this block
                    # P0 = [f | i], P1 = [o | g] (by 64-partition halves)
                    s_fi = sp.tile([128, blk_px], f32, tag="sfi", name=f"sfi{bi}")
                    nc.scalar.activation(
                        s_fi[:], P0[:], AF.Sigmoid, bias=b_sb[:, 0:1]
                    )
                    so = sp.tile([64, blk_px], f32, tag="so", name=f"so{bi}")
                    nc.scalar.activation(
                        so[:], P1[0:64, :], AF.Sigmoid, bias=b_sb[0:64, 1:2]
                    )
                    cg = sp.tile([128, blk_px], f32, tag="cg", name=f"cg{bi}")
                    nc.scalar.activation(
                        cg[64:128, :], P1[64:128, :], AF.Tanh, bias=b_sb[64:128, 1:2]
                    )
                    nc.scalar.dma_start(
                        cg[0:64, :].rearrange("c (y x) -> c y x", x=W),
                        c_d[b, :, y0 : y0 + rpb, :],
                    )
                    prod = sp.tile([128, blk_px], f32, tag="prod", name=f"prod{bi}")
                    nc.vector.tensor_mul(prod[:], s_fi[:], cg[:])
                    ig = sp.tile([64, blk_px], f32, tag="ig", name=f"ig{bi}")
                    nc.gpsimd.dma_start(ig[:], prod[64:128, :])
                    cn = sp.tile([64, blk_px], f32, tag="cn", name=f"cn{bi}")
                    nc.vector.tensor_add(cn[:], prod[0:64, :], ig[:])
                    tch = sp.tile([64, blk_px], f32, tag="tch", name=f"tch{bi}")
                    nc.scalar.activation(tch[:], cn[:], AF.Tanh)
                    hh = sp.tile([64, blk_px], f32, tag="hh", name=f"hh{bi}")
                    nc.vector.tensor_mul(hh[:], so[:], tch[:])
                    nc.sync.dma_start(
                        cn_d[b, :, y0 : y0 + rpb, :],
                        cn[:].rearrange("c (y x) -> c y x", x=W),
                    )
                    nc.sync.dma_start(
                        hn_d[b, :, y0 : y0 + rpb, :],
                        hh[:].rearrange("c (y x) -> c y x", x=W),
                    )
                    bi += 1

    nc.compile()
    return nc


def get_program():
    if "nc" not in _CACHE:
        _CACHE["nc"] = _build_program()
    return _CACHE["nc"]


def _prep_host(inputs):
    """Pack weights/biases; convert x/h to bf16; build per-core input maps."""
    x = np.asarray(inputs["x"], np.float32)
    h = np.asarray(inputs["hidden_state"], np.float32)
    c = np.asarray(inputs["cell_state"], np.float32)

    # gate column order [f, i, o, g] -> chunk0=[f,i], chunk1=[o,g]
    gx = [inputs["w_xf"], inputs["w_xi"], inputs["w_xo"], inputs["w_xg"]]
    gh = [inputs["w_hf"], inputs["w_hi"], inputs["w_ho"], inputs["w_hg"]]
    wx = np.stack([np.asarray(a, np.float32) for a in gx])  # [4, HC, C_IN, 3, 3]
    wh = np.stack([np.asarray(a, np.float32) for a in gh])  # [4, HC, HC, 3, 3]
    # -> [c, dy, dx, gate, o] -> [c, 9, 256]
    wxc = np.transpose(wx, (2, 3, 4, 0, 1)).reshape(C_IN, 9, 4 * HC)
    whc = np.transpose(wh, (2, 3, 4, 0, 1)).reshape(HC, 9, 4 * HC)
    wcat = np.concatenate([wxc, whc], 0).reshape(CTOT, 9 * 4 * HC).astype(BF16)

    bf = np.asarray(inputs["b_xf"], np.float32) + np.asarray(inputs["b_hf"], np.float32)
    bi = np.asarray(inputs["b_xi"], np.float32) + np.asarray(inputs["b_hi"], np.float32)
    bo = np.asarray(inputs["b_xo"], np.float32) + np.asarray(inputs["b_ho"], np.float32)
    bg = np.asarray(inputs["b_xg"], np.float32) + np.asarray(inputs["b_hg"], np.float32)
    bias = np.stack(
        [np.concatenate([bf, bi]), np.concatenate([bo, bg])], axis=1
    ).astype(np.float32)  # [128, 2]

    xb = x.astype(BF16)
    hb = h.astype(BF16)

    in_maps = []
    for i in range(N_CORES):
        s = slice(i * B_LOC, (i + 1) * B_LOC)
        in_maps.append(
            {
                "x": xb[s],
                "h": hb[s],
                "c": c[s],
                "w": wcat,
                "bias": bias,
            }
        )
    return in_maps


def run(inputs, trace=False, trace_kwargs=None):
    from concourse.bass_utils import run_bass_kernel_spmd

    nc = get_program()
    in_maps = _prep_host(inputs)
    res = run_bass_kernel_spmd(
        nc,
        in_maps,
        list(range(N_CORES)),
        trace=trace,
        **(trace_kwargs or {}),
    )
    h_new = np.concatenate([r["hn"] for r in res.results], 0).astype(np.float32)
    c_new = np.concatenate([r["cn"] for r in res.results], 0).astype(np.float32)
    return (h_new, c_new), res


def kernel(**inputs):
    (h_new, c_new), _ = run(inputs, trace=False)
    return (h_new, c_new)
